# revision 68
# baseline (speedup 1.0000x reference)
"""Masked-attention + log-softmax kernel for Trainium2 (Bass/Tile).

Computes, per batch element b (one NeuronCore each, 8 cores):
    s   = (Q @ K^T) / sqrt(128)          [2048, 2048]
    s_m = where(mask, -inf, s)
    out_attn = log_softmax(s_m, axis=-1) = s_m - ln(sum_k exp(s_m))
    out      = softmax(s_m) @ K

No row-max subtraction is needed: s ~ N(0,1), |s| < ~7, so exp(s) is far
from fp32 overflow and sum_k exp(s) < 2^22.

Per-core pipeline (q-tile = 128 query rows, 16 q-tiles):
  PE : S = Q^T.T @ K^T (fp16 operands, fp32 PSUM); PE transposes of the
       masked scores s_m (fp16); PV matmul with K augmented by a ones
       column whose PSUM column 128 accumulates r = sum_k exp(s_m).
  DVE: t = mask_u8 * 0xFC00 (u16) -> bitcast fp16 {0.0, -inf} (exact);
       s_m = S_psum + t (fp16 out).
  ACT: e^T = Exp(s^T_psum) (fuses the PSUM drain with exp);
       out_attn = Identity(s_m + bias(-ln r)) (per-partition bias path);
       out = Copy(pv * scale(1/r)); ln r.
All ACT funcs (Exp/Ln/Identity/Copy) live in one table set
(natural_log_exp_and_others); Bacc subclass pins it to avoid table swaps.
"""

import math
import sys

import numpy as np

for _p in ("/opt/trn_rl_repo",):
    if _p not in sys.path:
        sys.path.insert(0, _p)

from contextlib import ExitStack

import bass_rust as _bass_rust
import concourse.bacc as bacc
import concourse.bass as bass
import concourse.mybir as mybir
import concourse.tile as tile
from concourse.bass_utils import run_bass_kernel_spmd
from concourse.hw_specs import get_activation_tables
from concourse.masks import make_identity

F32 = mybir.dt.float32
F16 = mybir.dt.float16
U8 = mybir.dt.uint8
U16 = mybir.dt.uint16
AF = mybir.ActivationFunctionType
OP = mybir.AluOpType

P = 128
D = 128
B = 8
N_CORES = 8
INV_TEMP = 1.0 / math.sqrt(D)
F16_NINF_BITS = 0xFC00

# How many q-tiles the front (QK matmul + mask) runs ahead of the back half
# (transpose/exp/PV/outputs) in the per-engine instruction streams.
STAGGER = 2


class _Bacc(bacc.Bacc):
    def insert_act_table_loads(self):
        has_activation = any(
            isinstance(i, mybir.InstActivation)
            for b in self.main_func.blocks
            for i in b.instructions
        )
        if not has_activation:
            return
        tables = list(get_activation_tables(self.m.arch).items())
        # Steer every activation to the one set containing Exp AND Ln AND
        # Identity/Copy so the kernel needs exactly one ACT_TABLE_LOAD (the
        # default greedy choice alternates exp_and_others <-> natural_log,
        # costing ~2.7us per swap, twice per q-tile). act_func_set_id is the
        # INDEX into this list (must match act_info.json), so the order must
        # be preserved — instead remove our funcs from every other set.
        mine = {AF.Exp, AF.Ln, AF.Identity, AF.Copy}
        if any(n == "natural_log_exp_and_others" and mine <= fs for n, fs in tables):
            tables = [
                (n, fs if n == "natural_log_exp_and_others" else fs - mine)
                for n, fs in tables
            ]
        _bass_rust.insert_act_table_loads(self, tables)


def _emit(ctx: ExitStack, tc: "tile.TileContext", q_d, k_d, m_d, o0_d, o1_d, lq, lk):
    nc = tc.nc
    qt_n = lq // P  # q-tiles
    kb_n = lk // P  # 128-wide k blocks
    kc_w = 1024  # S psum chunk width (2 banks)
    kc_n = lk // kc_w
    OUTF_BATCH = 4

    const = ctx.enter_context(tc.tile_pool(name="const", bufs=1))
    prep = ctx.enter_context(tc.tile_pool(name="prep", bufs=4))
    mask_p = ctx.enter_context(tc.tile_pool(name="maskp", bufs=3))
    t_p = ctx.enter_context(tc.tile_pool(name="tp", bufs=2))
    sm_p = ctx.enter_context(tc.tile_pool(name="smp", bufs=STAGGER + 2))
    et_p = ctx.enter_context(tc.tile_pool(name="etp", bufs=4))
    oa_p = ctx.enter_context(tc.tile_pool(name="oap", bufs=3))
    outf_p = ctx.enter_context(tc.tile_pool(name="outfp", bufs=2))
    vec_p = ctx.enter_context(tc.tile_pool(name="vecp", bufs=3))
    s_ps = ctx.enter_context(tc.tile_pool(name="sps", bufs=2, space="PSUM"))
    st_ps = ctx.enter_context(tc.tile_pool(name="stps", bufs=3, space="PSUM"))
    pv_ps = ctx.enter_context(tc.tile_pool(name="pvps", bufs=1, space="PSUM"))

    ident32 = const.tile([P, P], F32, tag="ident32")
    make_identity(nc, ident32[:])
    ident16 = const.tile([P, P], F16, tag="ident16")
    nc.vector.tensor_copy(ident16[:], ident32[:])

    qt_all = const.tile([P, lq], F16, tag="qt")  # Q^T / sqrt(D), [d, q]
    kt_all = const.tile([P, lk], F16, tag="kt")  # K^T, [d, k]
    kaug = const.tile([P, kb_n * (D + 1)], F16, tag="kaug")  # [k-blk, D+1]

    # Warm the PE HAM clock gate (~3.4us of activity flips 1.2 -> 2.4 GHz)
    # with dummy matmuls while the K DMAs are still in flight.
    warm16 = const.tile([P, P], F16, tag="warm16")
    nc.vector.memset(warm16[:], 0.0)
    for _w in range(16):
        stw = st_ps.tile([P, 8 * P], F16, tag="st8", name="stw")
        nc.tensor.matmul(
            stw[:, 0 : 2 * P].bitcast(F32),
            lhsT=warm16[:],
            rhs=warm16[:],
            start=True,
            stop=True,
        )

    # K first: the first QK matmul needs all of K^T. Q tiles are prepped
    # lazily inside the main loop so QK(0) starts as soon as possible.
    for i4 in range(kb_n // 4):
        kf = prep.tile([P, 4, D], F32, tag="kstg")
        nc.sync.dma_start(
            kf[:],
            k_d[i4 * 4 * P : (i4 + 1) * 4 * P, :].rearrange("(j p) c -> p j c", p=P),
        )
        for j in range(4):
            i = i4 * 4 + j
            stp = st_ps.tile([P, 8 * P], F16, tag="st8", name="stp")
            ps = stp[:, 0 : 2 * P].bitcast(F32)
            nc.tensor.transpose(ps, kf[:, j, :], ident32[:])
            nc.vector.tensor_copy(kt_all[:, i * P : (i + 1) * P], ps)
            nc.vector.tensor_copy(kaug[:, i * (D + 1) : i * (D + 1) + D], kf[:, j, :])
            nc.vector.memset(kaug[:, i * (D + 1) + D : (i + 1) * (D + 1)], 1.0)

    def prep_q(i4):
        qf = prep.tile([P, 4, D], F32, tag="qstg")
        nc.sync.dma_start(
            qf[:],
            q_d[i4 * 4 * P : (i4 + 1) * 4 * P, :].rearrange("(j p) c -> p j c", p=P),
        )
        for j in range(4):
            i = i4 * 4 + j
            stp = st_ps.tile([P, 8 * P], F16, tag="st8", name="stp")
            ps = stp[:, 0 : 2 * P].bitcast(F32)
            nc.tensor.transpose(ps, qf[:, j, :], ident32[:])
            nc.vector.tensor_scalar(
                qt_all[:, i * P : (i + 1) * P], ps, INV_TEMP, None, op0=OP.mult
            )

    sm_tiles = {}
    outf_tiles = {}

    def front(qi):
        mt = mask_p.tile([P, lk], U8, tag="mask")
        nc.gpsimd.dma_start(mt[:], m_d[qi * P : (qi + 1) * P, :])
        sm_t = sm_p.tile([P, lk], F16, tag="sm")
        for c in range(kc_n):
            s_t = s_ps.tile([P, kc_w], F32, tag="s")
            for h in range(kc_w // 512):
                lo = c * kc_w + h * 512
                nc.tensor.matmul(
                    s_t[:, h * 512 : (h + 1) * 512],
                    lhsT=qt_all[:, qi * P : (qi + 1) * P],
                    rhs=kt_all[:, lo : lo + 512],
                    start=True,
                    stop=True,
                )
            # Fused mask + PSUM drain: (mask_u8 * -1e9) + S in fp32, written
            # as fp16 — masked lanes overflow the fp16 range and round to
            # exactly -inf; unmasked lanes add 0. One DVE pass replaces the
            # old t-gen + tensor_tensor pair. Immediate scalar (finite, so it
            # survives BIR JSON; AP-operand form would be ~6x slower).
            nc.vector.scalar_tensor_tensor(
                sm_t[:, c * kc_w : (c + 1) * kc_w],
                mt[:, c * kc_w : (c + 1) * kc_w],
                -1e9,
                s_t[:],
                op0=OP.mult,
                op1=OP.add,
            )
        sm_tiles[qi] = sm_t

    def back(qj):
        sm_t = sm_tiles.pop(qj)
        ets = []
        for half in range(kb_n // 8):
            stp = st_ps.tile([P, 8 * P], F16, tag="st8")
            for jj in range(8):
                j = half * 8 + jj
                nc.tensor.transpose(
                    stp[:, jj * P : (jj + 1) * P],
                    sm_t[:, j * P : (j + 1) * P],
                    ident16[:],
                )
            et = et_p.tile([P, 8 * P], F16, tag="et")
            nc.scalar.activation(et[:], stp[:], AF.Exp)
            ets.append(et)
        pv = pv_ps.tile([P, D + 1], F32, tag="pv")
        for j in range(kb_n):
            nc.tensor.matmul(
                pv[:],
                lhsT=ets[j // 8][:, (j % 8) * P : (j % 8 + 1) * P],
                rhs=kaug[:, j * (D + 1) : (j + 1) * (D + 1)],
                start=(j == 0),
                stop=(j == kb_n - 1),
            )
        lnr = vec_p.tile([P, 1], F32, tag="lnr")
        nc.scalar.activation(lnr[:], pv[:, D : D + 1], AF.Ln)
        nlnr = vec_p.tile([P, 1], F32, tag="nlnr")
        nc.vector.tensor_scalar(nlnr[:], lnr[:], -1.0, None, op0=OP.mult)
        rinv = vec_p.tile([P, 1], F32, tag="rinv")
        nc.vector.reciprocal(rinv[:], pv[:, D : D + 1])
        # out tile: batch OUTF_BATCH q-tiles into one DMA
        slot = qj % OUTF_BATCH
        if slot == 0:
            outf_tiles[qj // OUTF_BATCH] = outf_p.tile(
                [P, OUTF_BATCH, D], F32, tag="outf", name="outf"
            )
        outf = outf_tiles[qj // OUTF_BATCH]
        nc.scalar.mul(outf[:, slot, :], pv[:, 0:D], rinv[:])
        if slot == OUTF_BATCH - 1:
            base = (qj - slot) * P
            nc.gpsimd.dma_start(
                o0_d[base : base + OUTF_BATCH * P, :].rearrange(
                    "(j p) c -> p j c", p=P
                ),
                outf[:],
            )
        oa = oa_p.tile([P, lk], F32, tag="oa")
        # Split out_attn columns between DVE (broadcast-operand add) and the
        # ACT per-partition bias path to balance the two engines.
        OA_DVE = 768
        nc.vector.tensor_tensor(
            oa[:, 0:OA_DVE],
            sm_t[:, 0:OA_DVE],
            nlnr[:].broadcast_to([P, OA_DVE]),
            op=OP.add,
        )
        nc.scalar.activation(
            oa[:, OA_DVE:lk], sm_t[:, OA_DVE:lk], AF.Identity, bias=nlnr[:]
        )
        # 1-in-4 out_attn stores (and the drain-tail ones) ride the GPSIMD
        # SWDGE ring to relieve the sync HWDGE ring (~16MB stream).
        oa_eng = nc.gpsimd if qj % 4 == 1 else nc.sync
        oa_eng.dma_start(o1_d[qj * P : (qj + 1) * P, :], oa[:])

    for qi in range(qt_n + STAGGER):
        if qi < qt_n:
            if qi % 4 == 0:
                prep_q(qi // 4)
            front(qi)
        if qi >= STAGGER:
            back(qi - STAGGER)


def build_nc(lq=2048, lk=2048, compile=True):
    nc = _Bacc()
    q_d = nc.declare_dram_parameter("q", [lq, D], F32, isOutput=False)
    k_d = nc.declare_dram_parameter("k", [lk, D], F32, isOutput=False)
    m_d = nc.declare_dram_parameter("m", [lq, lk], U8, isOutput=False)
    o0_d = nc.declare_dram_parameter("o0", [lq, D], F32, isOutput=True)
    o1_d = nc.declare_dram_parameter("o1", [lq, lk], F32, isOutput=True)
    with tile.TileContext(nc, pool_alloc_mode="queue") as tc, ExitStack() as ctx:
        _emit(ctx, tc, q_d, k_d, m_d, o0_d, o1_d, lq, lk)
    if compile:
        nc.compile()
    return nc


_NC_CACHE = {}


def _get_nc():
    if "nc" not in _NC_CACHE:
        _NC_CACHE["nc"] = build_nc()
    return _NC_CACHE["nc"]


def _run(output, context, mask, **spmd_kwargs):
    nc = _get_nc()
    in_maps = [
        {
            "q": np.ascontiguousarray(np.asarray(output[b], dtype=np.float32)),
            "k": np.ascontiguousarray(np.asarray(context[b], dtype=np.float32)),
            "m": np.ascontiguousarray(np.asarray(mask[b]).view(np.uint8)),
        }
        for b in range(B)
    ]
    res = run_bass_kernel_spmd(nc, in_maps, list(range(N_CORES)), **spmd_kwargs)
    out = np.stack([res.results[b]["o0"] for b in range(B)])
    out_attn = np.stack([res.results[b]["o1"] for b in range(B)])
    return (out, out_attn), res


def kernel(output, context, mask):
    (out, out_attn), _ = _run(output, context, mask)
    return out, out_attn


def _install_profile_hook(so_path="/opt/axon/libaxon_pjrt.so"):
    """Recreate the antenv.axon_hooks NTFF-profile shim this image lacks.

    Drives NRT profiling on the axon terminal via the C ABI of the
    injected PJRT plugin (same mechanism trn_boot would install).
    """
    import contextlib
    import ctypes
    import types

    if "antenv.axon_hooks" in sys.modules:
        return
    lib = ctypes.CDLL(so_path)
    if not hasattr(lib, "axon_start_nrt_profile"):
        return
    lib.axon_start_nrt_profile.argtypes = [
        ctypes.POINTER(ctypes.c_int64),
        ctypes.c_size_t,
    ]
    lib.axon_start_nrt_profile.restype = ctypes.c_int64
    lib.axon_stop_nrt_profile.argtypes = [ctypes.c_char_p]
    lib.axon_stop_nrt_profile.restype = ctypes.c_int64

    @contextlib.contextmanager
    def _hook(output_dir, device_ids):
        import jax

        jax.devices()
        if device_ids:
            ids = (ctypes.c_int64 * len(device_ids))(*device_ids)
            rc = lib.axon_start_nrt_profile(ids, len(device_ids))
        else:
            rc = lib.axon_start_nrt_profile(None, 0)
        if rc != 0:
            raise RuntimeError(f"axon_start_nrt_profile rc={rc}")
        try:
            yield
        finally:
            n = lib.axon_stop_nrt_profile(str(output_dir).encode())
            print(f"profile: {n} file(s) written to {output_dir}", file=sys.stderr)

    mod = types.ModuleType("antenv.axon_hooks")
    mod.get_axon_ntff_profile_hook = lambda: _hook
    mod.set_axon_ntff_profile_hook = lambda h: None
    sys.modules["antenv.axon_hooks"] = mod
    import antenv

    antenv.axon_hooks = mod


def kernel_profiled(output, context, mask, **kw):
    _install_profile_hook()
    import concourse.bass_utils as bu

    bu.upload_artifacts = lambda tmpdir: f"local://{tmpdir}"
    return _run(output, context, mask, trace=True, **kw)


# revision 69
# speedup vs baseline: 1.1208x; 1.1208x over previous
"""Masked-attention + log-softmax kernel for Trainium2 (Bass/Tile).

Computes, per batch element b (one NeuronCore each, 8 cores):
    s   = (Q @ K^T) / sqrt(128)          [2048, 2048]
    s_m = where(mask, -inf, s)
    out_attn = log_softmax(s_m, axis=-1) = s_m - ln(sum_k exp(s_m))
    out      = softmax(s_m) @ K

No row-max subtraction is needed: s ~ N(0,1), |s| < ~7, so exp(s) is far
from fp32 overflow and sum_k exp(s) < 2^22.

Per-core pipeline (q-tile = 128 query rows, 16 q-tiles):
  PE : S = Q^T.T @ K^T (fp16 operands, fp32 PSUM); PE transposes of the
       masked scores s_m (fp16); PV matmul with K augmented by a ones
       column whose PSUM column 128 accumulates r = sum_k exp(s_m).
  DVE: t = mask_u8 * 0xFC00 (u16) -> bitcast fp16 {0.0, -inf} (exact);
       s_m = S_psum + t (fp16 out).
  ACT: e^T = Exp(s^T_psum) (fuses the PSUM drain with exp);
       out_attn = Identity(s_m + bias(-ln r)) (per-partition bias path);
       out = Copy(pv * scale(1/r)); ln r.
All ACT funcs (Exp/Ln/Identity/Copy) live in one table set
(natural_log_exp_and_others); Bacc subclass pins it to avoid table swaps.
"""

import math
import sys

import numpy as np

for _p in ("/opt/trn_rl_repo",):
    if _p not in sys.path:
        sys.path.insert(0, _p)

from contextlib import ExitStack

import bass_rust as _bass_rust
import concourse.bacc as bacc
import concourse.bass as bass
import concourse.mybir as mybir
import concourse.tile as tile
from concourse.bass_utils import run_bass_kernel_spmd
from concourse.hw_specs import get_activation_tables
from concourse.masks import make_identity

F32 = mybir.dt.float32
F16 = mybir.dt.float16
U8 = mybir.dt.uint8
U16 = mybir.dt.uint16
AF = mybir.ActivationFunctionType
OP = mybir.AluOpType

P = 128
D = 128
B = 8
N_CORES = 8
INV_TEMP = 1.0 / math.sqrt(D)
F16_NINF_BITS = 0xFC00

# How many q-tiles the front (QK matmul + mask) runs ahead of the back half
# (transpose/exp/PV/outputs) in the per-engine instruction streams.
STAGGER = 2


class _Bacc(bacc.Bacc):
    def insert_act_table_loads(self):
        has_activation = any(
            isinstance(i, mybir.InstActivation)
            for b in self.main_func.blocks
            for i in b.instructions
        )
        if not has_activation:
            return
        tables = list(get_activation_tables(self.m.arch).items())
        # Steer every activation to the one set containing Exp AND Ln AND
        # Identity/Copy so the kernel needs exactly one ACT_TABLE_LOAD (the
        # default greedy choice alternates exp_and_others <-> natural_log,
        # costing ~2.7us per swap, twice per q-tile). act_func_set_id is the
        # INDEX into this list (must match act_info.json), so the order must
        # be preserved — instead remove our funcs from every other set.
        mine = {AF.Exp, AF.Ln, AF.Identity, AF.Copy}
        if any(n == "natural_log_exp_and_others" and mine <= fs for n, fs in tables):
            tables = [
                (n, fs if n == "natural_log_exp_and_others" else fs - mine)
                for n, fs in tables
            ]
        _bass_rust.insert_act_table_loads(self, tables)


def _emit(ctx: ExitStack, tc: "tile.TileContext", q_d, k_d, m_d, o0_d, o1_d, lq, lk):
    nc = tc.nc
    qt_n = lq // P  # q-tiles
    kb_n = lk // P  # 128-wide k blocks
    kc_w = 1024  # S psum chunk width (2 banks)
    kc_n = lk // kc_w
    OUTF_BATCH = 4

    const = ctx.enter_context(tc.tile_pool(name="const", bufs=1))
    prep = ctx.enter_context(tc.tile_pool(name="prep", bufs=4))
    mask_p = ctx.enter_context(tc.tile_pool(name="maskp", bufs=3))
    t_p = ctx.enter_context(tc.tile_pool(name="tp", bufs=2))
    sm_p = ctx.enter_context(tc.tile_pool(name="smp", bufs=STAGGER + 2))
    et_p = ctx.enter_context(tc.tile_pool(name="etp", bufs=4))
    oa_p = ctx.enter_context(tc.tile_pool(name="oap", bufs=3))
    outf_p = ctx.enter_context(tc.tile_pool(name="outfp", bufs=2))
    vec_p = ctx.enter_context(tc.tile_pool(name="vecp", bufs=3))
    s_ps = ctx.enter_context(tc.tile_pool(name="sps", bufs=2, space="PSUM"))
    st_ps = ctx.enter_context(tc.tile_pool(name="stps", bufs=3, space="PSUM"))
    pv_ps = ctx.enter_context(tc.tile_pool(name="pvps", bufs=1, space="PSUM"))

    ident32 = const.tile([P, P], F32, tag="ident32")
    make_identity(nc, ident32[:])
    ident16 = const.tile([P, P], F16, tag="ident16")
    nc.vector.tensor_copy(ident16[:], ident32[:])

    qt_all = const.tile([P, lq], F16, tag="qt")  # Q^T / sqrt(D), [d, q]
    kt_all = const.tile([P, lk], F16, tag="kt")  # K^T, [d, k]
    kaug = const.tile([P, kb_n * (D + 1)], F16, tag="kaug")  # [k-blk, D+1]

    # Warm the PE HAM clock gate (~3.4us of activity flips 1.2 -> 2.4 GHz)
    # with dummy matmuls while the K DMAs are still in flight.
    warm16 = const.tile([P, P], F16, tag="warm16")
    nc.vector.memset(warm16[:], 0.0)
    for _w in range(16):
        stw = st_ps.tile([P, 8 * P], F16, tag="st8", name="stw")
        nc.tensor.matmul(
            stw[:, 0 : 2 * P].bitcast(F32),
            lhsT=warm16[:],
            rhs=warm16[:],
            start=True,
            stop=True,
        )

    # K first: the first QK matmul needs all of K^T. Q tiles are prepped
    # lazily inside the main loop so QK(0) starts as soon as possible.
    for i4 in range(kb_n // 4):
        kf = prep.tile([P, 4, D], F32, tag="kstg")
        nc.sync.dma_start(
            kf[:],
            k_d[i4 * 4 * P : (i4 + 1) * 4 * P, :].rearrange("(j p) c -> p j c", p=P),
        )
        for j in range(4):
            i = i4 * 4 + j
            stp = st_ps.tile([P, 8 * P], F16, tag="st8", name="stp")
            ps = stp[:, 0 : 2 * P].bitcast(F32)
            nc.tensor.transpose(ps, kf[:, j, :], ident32[:])
            nc.vector.tensor_copy(kt_all[:, i * P : (i + 1) * P], ps)
            nc.vector.tensor_copy(kaug[:, i * (D + 1) : i * (D + 1) + D], kf[:, j, :])
            nc.vector.memset(kaug[:, i * (D + 1) + D : (i + 1) * (D + 1)], 1.0)

    def prep_q(i4):
        qf = prep.tile([P, 4, D], F32, tag="qstg")
        nc.sync.dma_start(
            qf[:],
            q_d[i4 * 4 * P : (i4 + 1) * 4 * P, :].rearrange("(j p) c -> p j c", p=P),
        )
        for j in range(4):
            i = i4 * 4 + j
            stp = st_ps.tile([P, 8 * P], F16, tag="st8", name="stp")
            ps = stp[:, 0 : 2 * P].bitcast(F32)
            nc.tensor.transpose(ps, qf[:, j, :], ident32[:])
            nc.vector.tensor_scalar(
                qt_all[:, i * P : (i + 1) * P], ps, INV_TEMP, None, op0=OP.mult
            )

    sm_tiles = {}
    outf_tiles = {}

    def front(qi):
        mt = mask_p.tile([P, lk], U8, tag="mask")
        nc.gpsimd.dma_start(mt[:], m_d[qi * P : (qi + 1) * P, :])
        sm_t = sm_p.tile([P, lk], F16, tag="sm")
        for c in range(kc_n):
            s_t = s_ps.tile([P, kc_w], F32, tag="s")
            for h in range(kc_w // 512):
                lo = c * kc_w + h * 512
                nc.tensor.matmul(
                    s_t[:, h * 512 : (h + 1) * 512],
                    lhsT=qt_all[:, qi * P : (qi + 1) * P],
                    rhs=kt_all[:, lo : lo + 512],
                    start=True,
                    stop=True,
                )
            # Fused mask + PSUM drain: (mask_u8 * -1e9) + S in fp32, written
            # as fp16 — masked lanes overflow the fp16 range and round to
            # exactly -inf; unmasked lanes add 0. One DVE pass replaces the
            # old t-gen + tensor_tensor pair. Immediate scalar (finite, so it
            # survives BIR JSON; AP-operand form would be ~6x slower).
            nc.vector.scalar_tensor_tensor(
                sm_t[:, c * kc_w : (c + 1) * kc_w],
                mt[:, c * kc_w : (c + 1) * kc_w],
                -1e9,
                s_t[:],
                op0=OP.mult,
                op1=OP.add,
            )
        sm_tiles[qi] = sm_t

    def back(qj):
        sm_t = sm_tiles.pop(qj)
        ets = []
        for half in range(kb_n // 8):
            stp = st_ps.tile([P, 8 * P], F16, tag="st8")
            for jj in range(8):
                j = half * 8 + jj
                nc.tensor.transpose(
                    stp[:, jj * P : (jj + 1) * P],
                    sm_t[:, j * P : (j + 1) * P],
                    ident16[:],
                )
            et = et_p.tile([P, 8 * P], F16, tag="et")
            nc.scalar.activation(et[:], stp[:], AF.Exp)
            ets.append(et)
        pv = pv_ps.tile([P, D + 1], F32, tag="pv")
        for j in range(kb_n):
            nc.tensor.matmul(
                pv[:],
                lhsT=ets[j // 8][:, (j % 8) * P : (j % 8 + 1) * P],
                rhs=kaug[:, j * (D + 1) : (j + 1) * (D + 1)],
                start=(j == 0),
                stop=(j == kb_n - 1),
            )
        lnr = vec_p.tile([P, 1], F32, tag="lnr")
        nc.scalar.activation(lnr[:], pv[:, D : D + 1], AF.Ln)
        nlnr = vec_p.tile([P, 1], F32, tag="nlnr")
        nc.vector.tensor_scalar(nlnr[:], lnr[:], -1.0, None, op0=OP.mult)
        rinv = vec_p.tile([P, 1], F32, tag="rinv")
        nc.vector.reciprocal(rinv[:], pv[:, D : D + 1])
        # out tile: batch OUTF_BATCH q-tiles into one DMA
        slot = qj % OUTF_BATCH
        if slot == 0:
            outf_tiles[qj // OUTF_BATCH] = outf_p.tile(
                [P, OUTF_BATCH, D], F32, tag="outf", name="outf"
            )
        outf = outf_tiles[qj // OUTF_BATCH]
        nc.scalar.mul(outf[:, slot, :], pv[:, 0:D], rinv[:])
        if slot == OUTF_BATCH - 1:
            base = (qj - slot) * P
            nc.gpsimd.dma_start(
                o0_d[base : base + OUTF_BATCH * P, :].rearrange(
                    "(j p) c -> p j c", p=P
                ),
                outf[:],
            )
        oa = oa_p.tile([P, lk], F32, tag="oa")
        # Split out_attn columns between DVE (broadcast-operand add) and the
        # ACT per-partition bias path to balance the two engines.
        OA_DVE = 256
        nc.vector.tensor_tensor(
            oa[:, 0:OA_DVE],
            sm_t[:, 0:OA_DVE],
            nlnr[:].broadcast_to([P, OA_DVE]),
            op=OP.add,
        )
        nc.scalar.activation(
            oa[:, OA_DVE:lk], sm_t[:, OA_DVE:lk], AF.Identity, bias=nlnr[:]
        )
        # 1-in-4 out_attn stores (and the drain-tail ones) ride the GPSIMD
        # SWDGE ring to relieve the sync HWDGE ring (~16MB stream).
        oa_eng = nc.gpsimd if qj % 4 == 1 else nc.sync
        oa_eng.dma_start(o1_d[qj * P : (qj + 1) * P, :], oa[:])

    for qi in range(qt_n + STAGGER):
        if qi < qt_n:
            if qi % 4 == 0:
                prep_q(qi // 4)
            front(qi)
        if qi >= STAGGER:
            back(qi - STAGGER)


def build_nc(lq=2048, lk=2048, compile=True):
    nc = _Bacc()
    q_d = nc.declare_dram_parameter("q", [lq, D], F32, isOutput=False)
    k_d = nc.declare_dram_parameter("k", [lk, D], F32, isOutput=False)
    m_d = nc.declare_dram_parameter("m", [lq, lk], U8, isOutput=False)
    o0_d = nc.declare_dram_parameter("o0", [lq, D], F32, isOutput=True)
    o1_d = nc.declare_dram_parameter("o1", [lq, lk], F32, isOutput=True)
    with tile.TileContext(nc, pool_alloc_mode="queue") as tc, ExitStack() as ctx:
        _emit(ctx, tc, q_d, k_d, m_d, o0_d, o1_d, lq, lk)
    if compile:
        nc.compile()
    return nc


_NC_CACHE = {}


def _get_nc():
    if "nc" not in _NC_CACHE:
        _NC_CACHE["nc"] = build_nc()
    return _NC_CACHE["nc"]


def _run(output, context, mask, **spmd_kwargs):
    nc = _get_nc()
    in_maps = [
        {
            "q": np.ascontiguousarray(np.asarray(output[b], dtype=np.float32)),
            "k": np.ascontiguousarray(np.asarray(context[b], dtype=np.float32)),
            "m": np.ascontiguousarray(np.asarray(mask[b]).view(np.uint8)),
        }
        for b in range(B)
    ]
    res = run_bass_kernel_spmd(nc, in_maps, list(range(N_CORES)), **spmd_kwargs)
    out = np.stack([res.results[b]["o0"] for b in range(B)])
    out_attn = np.stack([res.results[b]["o1"] for b in range(B)])
    return (out, out_attn), res


def kernel(output, context, mask):
    (out, out_attn), _ = _run(output, context, mask)
    return out, out_attn


def _install_profile_hook(so_path="/opt/axon/libaxon_pjrt.so"):
    """Recreate the antenv.axon_hooks NTFF-profile shim this image lacks.

    Drives NRT profiling on the axon terminal via the C ABI of the
    injected PJRT plugin (same mechanism trn_boot would install).
    """
    import contextlib
    import ctypes
    import types

    if "antenv.axon_hooks" in sys.modules:
        return
    lib = ctypes.CDLL(so_path)
    if not hasattr(lib, "axon_start_nrt_profile"):
        return
    lib.axon_start_nrt_profile.argtypes = [
        ctypes.POINTER(ctypes.c_int64),
        ctypes.c_size_t,
    ]
    lib.axon_start_nrt_profile.restype = ctypes.c_int64
    lib.axon_stop_nrt_profile.argtypes = [ctypes.c_char_p]
    lib.axon_stop_nrt_profile.restype = ctypes.c_int64

    @contextlib.contextmanager
    def _hook(output_dir, device_ids):
        import jax

        jax.devices()
        if device_ids:
            ids = (ctypes.c_int64 * len(device_ids))(*device_ids)
            rc = lib.axon_start_nrt_profile(ids, len(device_ids))
        else:
            rc = lib.axon_start_nrt_profile(None, 0)
        if rc != 0:
            raise RuntimeError(f"axon_start_nrt_profile rc={rc}")
        try:
            yield
        finally:
            n = lib.axon_stop_nrt_profile(str(output_dir).encode())
            print(f"profile: {n} file(s) written to {output_dir}", file=sys.stderr)

    mod = types.ModuleType("antenv.axon_hooks")
    mod.get_axon_ntff_profile_hook = lambda: _hook
    mod.set_axon_ntff_profile_hook = lambda h: None
    sys.modules["antenv.axon_hooks"] = mod
    import antenv

    antenv.axon_hooks = mod


def kernel_profiled(output, context, mask, **kw):
    _install_profile_hook()
    import concourse.bass_utils as bu

    bu.upload_artifacts = lambda tmpdir: f"local://{tmpdir}"
    return _run(output, context, mask, trace=True, **kw)
